# revision 30
# baseline (speedup 1.0000x reference)
"""Trainium2 Bass kernel for nn_AttentionLayer (B=8, H=W=64, C=256, D=128).

Strategy: data-parallel over batch B=8 across the 8 NeuronCores (attention is
independent per batch element). Per core, for its batch element's x [L=4096,
C=256]:

  phase 1: PE-transpose x tiles -> xT, project q^T,k^T [D, L] and v^T,
           then PE-transpose v^T -> v [L, D].
  phase 2 (per 512-wide Lq chunk):
      for each 128-row Lk tile:
        MM1: S^T chunk   = k_tile @ q_chunk^T            (PE, fp32r)
        exp: P~^T chunk  = exp(S^T chunk)                (ACT, ->fp32r)
        MM2: A~^T       += v_tile^T @ P~^T chunk         (PE, accumulate)
        MM3: denom      += ones^T  @ P~^T chunk          (PE, accumulate)
      scale = gamma / denom (DVE recip + broadcast), A = A~ * scale
      MM4: out = A @ Wlast ; out += x ; DMA out.

All matmuls run in float32r (full PE rate at moving-dim>=256, ~12.7 effective
mantissa bits) with fp32 PSUM accumulation. Softmax skips the max-subtraction:
logits are O(+-45) so exp stays comfortably inside fp32 range, and softmax is
shift-invariant so the result matches the reference.
"""

import numpy as np

import concourse.bass as bass
import concourse.mybir as mybir
import concourse.tile as tile
from concourse import bacc
from concourse.masks import make_identity
from concourse.bass_utils import run_bass_kernel_spmd

f32 = mybir.dt.float32
f32r = mybir.dt.float32r
AF = mybir.ActivationFunctionType
ALU = mybir.AluOpType

B, H, W, C, D = 8, 64, 64, 256, 128
L = H * W            # 4096
NT = L // 128        # 32 L-tiles of 128 rows
NCHUNK = L // 512    # 8 Lq chunks of 512
CK = C // 128        # 2 C-chunks


def _emit(nc, tc, ctx, nreps=1):
    x_d = nc.declare_dram_parameter("x", [L, C], f32, isOutput=False)
    wq_d = nc.declare_dram_parameter("Wq", [C, D], f32, isOutput=False)
    wk_d = nc.declare_dram_parameter("Wk", [C, D], f32, isOutput=False)
    wv_d = nc.declare_dram_parameter("Wv", [C, D], f32, isOutput=False)
    wl_d = nc.declare_dram_parameter("Wlast", [D, C], f32, isOutput=False)
    g_d = nc.declare_dram_parameter("gamma", [1], f32, isOutput=False)
    out_d = nc.declare_dram_parameter("out", [L, C], f32, isOutput=True)

    x_tiled = x_d[:].rearrange("(t p) c -> p t c", p=128)      # [128, NT, C]
    out_tiled = out_d[:].rearrange("(t p) c -> p t c", p=128)  # [128, NT, C]

    const = ctx.enter_context(tc.tile_pool(name="const", bufs=1))
    resident = ctx.enter_context(tc.tile_pool(name="resident", bufs=1))

    # --- constants -------------------------------------------------------
    identity = const.tile([128, 128], f32)
    make_identity(nc, identity[:])
    ones_f = const.tile([128, 1], f32)
    nc.vector.memset(ones_f[:], 1.0)
    ones_r = const.tile([128, 1], f32r)
    nc.vector.tensor_copy(out=ones_r[:], in_=ones_f[:])
    id1 = const.tile([1, 1], f32)
    nc.vector.memset(id1[:], 1.0)
    gamma_sb = const.tile([128, 1], f32)
    nc.sync.dma_start(out=gamma_sb[:], in_=g_d[:].to_broadcast((128, 1)))

    # weights: lhsT chunks [C128, D] for q/k/v, [D, C] for last
    w_r = {}
    for name, wd in (("q", wq_d), ("k", wk_d), ("v", wv_d)):
        wtmp = const.tile([128, CK, D], f32, name=f"wtmp_{name}")
        nc.sync.dma_start(out=wtmp[:], in_=wd[:].rearrange("(cc p) d -> p cc d", p=128))
        wr = const.tile([128, CK, D], f32r, name=f"w_{name}")
        nc.vector.tensor_copy(out=wr[:], in_=wtmp[:])
        w_r[name] = wr
    wl_tmp = const.tile([128, C], f32)
    nc.sync.dma_start(out=wl_tmp[:], in_=wl_d[:])
    wl_r = const.tile([128, C], f32r)
    nc.vector.tensor_copy(out=wl_r[:], in_=wl_tmp[:])

    if nreps == 1:
        _emit_body(nc, tc, const, resident, x_tiled, out_tiled,
                   identity, id1, ones_r, gamma_sb, w_r, wl_r)
    else:
        # dev-harness timing build: hardware loop re-running the identical
        # body (same inputs/outputs each iteration)
        with tc.For_i(0, nreps, 1):
            _emit_body(nc, tc, const, resident, x_tiled, out_tiled,
                       identity, id1, ones_r, gamma_sb, w_r, wl_r)


def _emit_body(nc, tc, const, resident, x_tiled, out_tiled,
               identity, id1, ones_r, gamma_sb, w_r, wl_r):
    # --- resident tensors ------------------------------------------------
    x_sb = resident.tile([128, NT, C], f32, tag="x_sb")      # 32 KB/part
    for s in range(4):
        nc.sync.dma_start(
            out=x_sb[:, s * 8:(s + 1) * 8, :], in_=x_tiled[:, s * 8:(s + 1) * 8, :]
        )
    qT_sb = resident.tile([128, L], f32r, tag="qT")          # 16 KB/part
    kT_sb = resident.tile([128, L], f32r, tag="kT")          # 16 KB/part
    v_sb = resident.tile([128, NT, D], f32r, tag="v")        # 16 KB/part

    # --- phase 1: transposes + projections -------------------------------
    with (
        tc.tile_pool(name="xt", bufs=2) as xtp,
        tc.tile_pool(name="vt", bufs=2) as vtp,
        tc.tile_pool(name="ps_tr", bufs=2, space="PSUM") as ps_tr,
        tc.tile_pool(name="ps_proj", bufs=2, space="PSUM") as ps_proj,
    ):
        for c in range(NCHUNK):
            cs = slice(c * 512, (c + 1) * 512)
            # x^T for this chunk: [128, CK, 512] (C-chunk on dim1)
            xt_c = xtp.tile([128, CK, 512], f32r)
            for cc in range(CK):
                ps = ps_tr.tile([128, 512], f32, tag="tr")
                for i in range(4):
                    t = 4 * c + i
                    nc.tensor.transpose(
                        ps[:, i * 128:(i + 1) * 128],
                        x_sb[:, t, cc * 128:(cc + 1) * 128], identity[:],
                    )
                nc.vector.tensor_copy(out=xt_c[:, cc, :], in_=ps[:])
            # q^T, k^T chunks
            for name, dstT in (("q", qT_sb), ("k", kT_sb)):
                ps = ps_proj.tile([128, 512], f32, tag="proj")
                for cc in range(CK):
                    nc.tensor.matmul(
                        ps[:], w_r[name][:, cc, :], xt_c[:, cc, :],
                        start=(cc == 0), stop=(cc == CK - 1),
                    )
                nc.vector.tensor_copy(out=dstT[:, cs], in_=ps[:])
            # v^T chunk then transpose into v [L-tile, D] blocks
            ps = ps_proj.tile([128, 512], f32, tag="proj")
            for cc in range(CK):
                nc.tensor.matmul(
                    ps[:], w_r["v"][:, cc, :], xt_c[:, cc, :],
                    start=(cc == 0), stop=(cc == CK - 1),
                )
            vt_c = vtp.tile([128, 512], f32)
            nc.vector.tensor_copy(out=vt_c[:], in_=ps[:])
            ps2 = ps_tr.tile([128, 512], f32, tag="tr")
            for i in range(4):
                nc.tensor.transpose(
                    ps2[:, i * 128:(i + 1) * 128],
                    vt_c[:, i * 128:(i + 1) * 128], identity[:],
                )
            nc.vector.tensor_copy(out=v_sb[:, 4 * c:4 * c + 4, :], in_=ps2[:])

    # --- phase 2: attention ----------------------------------------------
    NG = NT // 2  # 16 exp-groups of 2 Lk tiles
    with (
        tc.tile_pool(name="pexp", bufs=8) as pexp,
        tc.tile_pool(name="psum2p", bufs=4) as psum2p,
        tc.tile_pool(name="asb", bufs=2) as asb,
        tc.tile_pool(name="osb", bufs=2) as osb,
        tc.tile_pool(name="dsb", bufs=2) as dsb,
        tc.tile_pool(name="ps_s", bufs=2, space="PSUM") as ps_s,
        tc.tile_pool(name="ps_acc", bufs=1, space="PSUM") as ps_acc,
        tc.tile_pool(name="ps_den", bufs=1, space="PSUM") as ps_den,
        tc.tile_pool(name="ps_sc", bufs=1, space="PSUM") as ps_sc_p,
        tc.tile_pool(name="ps_out", bufs=1, space="PSUM") as ps_out,
    ):
        import os
        variant = os.environ.get("KVARIANT", "")
        if variant == "pefloor":
            pconst = const.tile([128, 2, 512], f32r, name="pconst")
            nc.vector.memset(pconst[:].bitcast(f32), 1.0)
        for c in range(NCHUNK):
            cs = slice(c * 512, (c + 1) * 512)
            acc = ps_acc.tile([128, 512], f32)
            den = ps_den.tile([1, 512], f32)

            def mm23_for(p2, ps2, g):
                for j in range(2):
                    lk = 2 * g + j
                    nc.tensor.matmul(
                        acc[:], v_sb[:, lk, :], p2[:, j, :],
                        start=(lk == 0), stop=(lk == NT - 1),
                        skip_group_check=True,
                    )
                if variant != "nomm3":
                    nc.tensor.matmul(
                        den[:], ones_r[:], ps2[:],
                        start=(g == 0), stop=(g == NG - 1),
                        skip_group_check=True,
                    )

            pipe = []
            for g in range(NG):
                s2 = ps_s.tile([128, 2, 512], f32)
                for j in range(2):
                    lk = 2 * g + j
                    nc.tensor.matmul(
                        s2[:, j, :], kT_sb[:, lk * 128:(lk + 1) * 128],
                        qT_sb[:, cs], start=True, stop=True,
                    )
                if variant == "pefloor":
                    p2 = pconst
                    snk = dsb.tile([128, 2], f32, tag="snk", name="snk")
                    nc.vector.tensor_copy(out=snk[:], in_=s2[:, :, 0])
                else:
                    p2 = pexp.tile([128, 2, 512], f32r)
                    nc.scalar.activation(out=p2[:], in_=s2[:], func=AF.Exp)
                # pre-sum the two 512-slices on DVE so the denominator
                # matmul streams half the columns
                ps2 = psum2p.tile([128, 512], f32r, tag="ps2", name="ps2")
                nc.vector.tensor_tensor(
                    out=ps2[:], in0=p2[:, 0, :].bitcast(f32),
                    in1=p2[:, 1, :].bitcast(f32), op=ALU.add,
                )
                pipe.append((p2, ps2, g))
                if len(pipe) > 2:
                    mm23_for(*pipe.pop(0))
            while pipe:
                mm23_for(*pipe.pop(0))

            # denominator row -> free dim of partition 0, then transpose to
            # per-partition scale columns
            tall = dsb.tile([1, 512], f32, tag="tall")
            nc.vector.tensor_copy(out=tall[:], in_=den[:])
            ps_sc = ps_sc_p.tile([128, 4], f32)
            for m in range(4):
                nc.tensor.transpose(
                    ps_sc[:, m:m + 1], tall[0:1, m * 128:(m + 1) * 128], id1[:]
                )
            sc_raw = dsb.tile([128, 4], f32, tag="scraw")
            nc.vector.tensor_copy(out=sc_raw[:], in_=ps_sc[:])
            sc = dsb.tile([128, 4], f32, tag="sc")
            nc.vector.reciprocal(out=sc[:], in_=sc_raw[:])
            nc.vector.tensor_scalar_mul(sc[:], sc[:], gamma_sb[:])

            # A~^T to SBUF (fp32r) for MM4
            a_sb = asb.tile([128, 512], f32r)
            nc.vector.tensor_copy(out=a_sb[:], in_=acc[:])

            o_sb = osb.tile([128, 4, C], f32)
            for m in range(4):
                t = 4 * c + m
                po = ps_out.tile([128, C], f32, tag="po")
                nc.tensor.matmul(
                    po[:], a_sb[:, m * 128:(m + 1) * 128], wl_r[:],
                    start=True, stop=True,
                )
                nc.vector.scalar_tensor_tensor(
                    out=o_sb[:, m, :], in0=po[:], scalar=sc[:, m:m + 1],
                    in1=x_sb[:, t, :], op0=ALU.mult, op1=ALU.add,
                )
            nc.sync.dma_start(
                out=out_tiled[:, 4 * c:4 * c + 4, :], in_=o_sb[:]
            )


_NC_CACHE = {}


def _build(nreps=1):
    """Build the Bass module; nreps>1 repeats the whole body (for marginal-
    time measurement in the dev harness — grading path uses nreps=1)."""
    if nreps not in _NC_CACHE:
        from contextlib import ExitStack

        nc = bacc.Bacc("TRN2", target_bir_lowering=False)
        with tile.TileContext(nc) as tc:
            with ExitStack() as ctx:
                _emit(nc, tc, ctx, nreps=nreps)
        nc.compile()
        _NC_CACHE[nreps] = nc
    return _NC_CACHE[nreps]


def kernel(x, Wq, Wk, Wv, Wlast, gamma):
    assert x.shape == (B, H, W, C), x.shape
    nc = _build()
    xf = np.ascontiguousarray(x, dtype=np.float32).reshape(B, L, C)
    in_maps = [
        {
            "x": xf[b],
            "Wq": np.ascontiguousarray(Wq, dtype=np.float32),
            "Wk": np.ascontiguousarray(Wk, dtype=np.float32),
            "Wv": np.ascontiguousarray(Wv, dtype=np.float32),
            "Wlast": np.ascontiguousarray(Wlast, dtype=np.float32),
            "gamma": np.ascontiguousarray(gamma, dtype=np.float32),
        }
        for b in range(B)
    ]
    res = run_bass_kernel_spmd(nc, in_maps, core_ids=list(range(B)))
    out = np.stack([res.results[b]["out"] for b in range(B)], axis=0)
    return out.reshape(B, H, W, C)


# revision 31
# speedup vs baseline: 1.1791x; 1.1791x over previous
"""Trainium2 Bass kernel for nn_AttentionLayer (B=8, H=W=64, C=256, D=128).

Strategy: data-parallel over batch B=8 across the 8 NeuronCores (attention is
independent per batch element). Per core, for its batch element's x [L=4096,
C=256]:

  phase 1: PE-transpose x tiles -> xT, project q^T,k^T [D, L] and v^T,
           then PE-transpose v^T -> v [L, D].
  phase 2 (per 512-wide Lq chunk):
      for each 128-row Lk tile:
        MM1: S^T chunk   = k_tile @ q_chunk^T            (PE, fp32r)
        exp: P~^T chunk  = exp(S^T chunk)                (ACT, ->fp32r)
        MM2: A~^T       += v_tile^T @ P~^T chunk         (PE, accumulate)
        MM3: denom      += ones^T  @ P~^T chunk          (PE, accumulate)
      scale = gamma / denom (DVE recip + broadcast), A = A~ * scale
      MM4: out = A @ Wlast ; out += x ; DMA out.

All matmuls run in float32r (full PE rate at moving-dim>=256, ~12.7 effective
mantissa bits) with fp32 PSUM accumulation. Softmax skips the max-subtraction:
logits are O(+-45) so exp stays comfortably inside fp32 range, and softmax is
shift-invariant so the result matches the reference.
"""

import numpy as np

import concourse.bass as bass
import concourse.mybir as mybir
import concourse.tile as tile
from concourse import bacc
from concourse.masks import make_identity
from concourse.bass_utils import run_bass_kernel_spmd

f32 = mybir.dt.float32
f32r = mybir.dt.float32r
AF = mybir.ActivationFunctionType
ALU = mybir.AluOpType

B, H, W, C, D = 8, 64, 64, 256, 128
L = H * W            # 4096
NT = L // 128        # 32 L-tiles of 128 rows
NCHUNK = L // 512    # 8 Lq chunks of 512
CK = C // 128        # 2 C-chunks


def _emit(nc, tc, ctx, nreps=1):
    x_d = nc.declare_dram_parameter("x", [L, C], f32, isOutput=False)
    wq_d = nc.declare_dram_parameter("Wq", [C, D], f32, isOutput=False)
    wk_d = nc.declare_dram_parameter("Wk", [C, D], f32, isOutput=False)
    wv_d = nc.declare_dram_parameter("Wv", [C, D], f32, isOutput=False)
    wl_d = nc.declare_dram_parameter("Wlast", [D, C], f32, isOutput=False)
    g_d = nc.declare_dram_parameter("gamma", [1], f32, isOutput=False)
    out_d = nc.declare_dram_parameter("out", [L, C], f32, isOutput=True)

    x_tiled = x_d[:].rearrange("(t p) c -> p t c", p=128)      # [128, NT, C]
    out_tiled = out_d[:].rearrange("(t p) c -> p t c", p=128)  # [128, NT, C]

    const = ctx.enter_context(tc.tile_pool(name="const", bufs=1))
    resident = ctx.enter_context(tc.tile_pool(name="resident", bufs=1))

    # --- constants -------------------------------------------------------
    identity = const.tile([128, 128], f32)
    make_identity(nc, identity[:])
    ones_f = const.tile([128, 1], f32)
    nc.vector.memset(ones_f[:], 1.0)
    ones_r = const.tile([128, 1], f32r)
    nc.vector.tensor_copy(out=ones_r[:], in_=ones_f[:])
    id1 = const.tile([1, 1], f32)
    nc.vector.memset(id1[:], 1.0)
    gamma_sb = const.tile([128, 1], f32)
    nc.sync.dma_start(out=gamma_sb[:], in_=g_d[:].to_broadcast((128, 1)))

    # weights: lhsT chunks [C128, D] for q/k/v, [D, C] for last
    w_r = {}
    for name, wd in (("q", wq_d), ("k", wk_d), ("v", wv_d)):
        wtmp = const.tile([128, CK, D], f32, name=f"wtmp_{name}")
        nc.sync.dma_start(out=wtmp[:], in_=wd[:].rearrange("(cc p) d -> p cc d", p=128))
        wr = const.tile([128, CK, D], f32r, name=f"w_{name}")
        nc.vector.tensor_copy(out=wr[:], in_=wtmp[:])
        w_r[name] = wr
    wl_tmp = const.tile([128, C], f32)
    nc.sync.dma_start(out=wl_tmp[:], in_=wl_d[:])
    wl_r = const.tile([128, C], f32r)
    nc.vector.tensor_copy(out=wl_r[:], in_=wl_tmp[:])

    if nreps == 1:
        _emit_body(nc, tc, const, resident, x_tiled, out_tiled,
                   identity, id1, ones_r, gamma_sb, w_r, wl_r)
    else:
        # dev-harness timing build: hardware loop re-running the identical
        # body (same inputs/outputs each iteration)
        with tc.For_i(0, nreps, 1):
            _emit_body(nc, tc, const, resident, x_tiled, out_tiled,
                       identity, id1, ones_r, gamma_sb, w_r, wl_r)


def _emit_body(nc, tc, const, resident, x_tiled, out_tiled,
               identity, id1, ones_r, gamma_sb, w_r, wl_r):
    # --- resident tensors ------------------------------------------------
    x_sb = resident.tile([128, NT, C], f32, tag="x_sb")      # 32 KB/part
    for s in range(4):
        nc.sync.dma_start(
            out=x_sb[:, s * 8:(s + 1) * 8, :], in_=x_tiled[:, s * 8:(s + 1) * 8, :]
        )
    qT_sb = resident.tile([128, L], f32r, tag="qT")          # 16 KB/part
    kT_sb = resident.tile([128, L], f32r, tag="kT")          # 16 KB/part
    v_sb = resident.tile([128, NT, D], f32r, tag="v")        # 16 KB/part

    # --- phase 1: transposes + projections -------------------------------
    with (
        tc.tile_pool(name="xt", bufs=2) as xtp,
        tc.tile_pool(name="vt", bufs=2) as vtp,
        tc.tile_pool(name="ps_tr", bufs=2, space="PSUM") as ps_tr,
        tc.tile_pool(name="ps_proj", bufs=2, space="PSUM") as ps_proj,
    ):
        for c in range(NCHUNK):
            cs = slice(c * 512, (c + 1) * 512)
            # x^T for this chunk: [128, CK, 512] (C-chunk on dim1)
            xt_c = xtp.tile([128, CK, 512], f32r)
            for cc in range(CK):
                ps = ps_tr.tile([128, 512], f32, tag="tr")
                for i in range(4):
                    t = 4 * c + i
                    nc.tensor.transpose(
                        ps[:, i * 128:(i + 1) * 128],
                        x_sb[:, t, cc * 128:(cc + 1) * 128], identity[:],
                    )
                nc.vector.tensor_copy(out=xt_c[:, cc, :], in_=ps[:])
            # q^T, k^T chunks
            for name, dstT in (("q", qT_sb), ("k", kT_sb)):
                ps = ps_proj.tile([128, 512], f32, tag="proj")
                for cc in range(CK):
                    nc.tensor.matmul(
                        ps[:], w_r[name][:, cc, :], xt_c[:, cc, :],
                        start=(cc == 0), stop=(cc == CK - 1),
                    )
                nc.vector.tensor_copy(out=dstT[:, cs], in_=ps[:])
            # v^T chunk then transpose into v [L-tile, D] blocks
            ps = ps_proj.tile([128, 512], f32, tag="proj")
            for cc in range(CK):
                nc.tensor.matmul(
                    ps[:], w_r["v"][:, cc, :], xt_c[:, cc, :],
                    start=(cc == 0), stop=(cc == CK - 1),
                )
            vt_c = vtp.tile([128, 512], f32)
            nc.vector.tensor_copy(out=vt_c[:], in_=ps[:])
            ps2 = ps_tr.tile([128, 512], f32, tag="tr")
            for i in range(4):
                nc.tensor.transpose(
                    ps2[:, i * 128:(i + 1) * 128],
                    vt_c[:, i * 128:(i + 1) * 128], identity[:],
                )
            nc.vector.tensor_copy(out=v_sb[:, 4 * c:4 * c + 4, :], in_=ps2[:])

    # --- phase 2: attention ----------------------------------------------
    NG = NT // 2  # 16 exp-groups of 2 Lk tiles
    with (
        tc.tile_pool(name="pexp", bufs=8) as pexp,
        tc.tile_pool(name="psum2p", bufs=4) as psum2p,
        tc.tile_pool(name="asb", bufs=2) as asb,
        tc.tile_pool(name="osb", bufs=2) as osb,
        tc.tile_pool(name="dsb", bufs=2) as dsb,
        tc.tile_pool(name="ps_s", bufs=4, space="PSUM") as ps_s,
        tc.tile_pool(name="ps_acc", bufs=1, space="PSUM") as ps_acc,
        tc.tile_pool(name="ps_den", bufs=1, space="PSUM") as ps_den,
        tc.tile_pool(name="ps_sc", bufs=1, space="PSUM") as ps_sc_p,
        tc.tile_pool(name="ps_out", bufs=1, space="PSUM") as ps_out,
    ):
        import os
        variant = os.environ.get("KVARIANT", "")
        if variant == "pefloor":
            pconst = const.tile([128, 2, 512], f32r, name="pconst")
            nc.vector.memset(pconst[:].bitcast(f32), 1.0)
        for c in range(NCHUNK):
            cs = slice(c * 512, (c + 1) * 512)
            acc = ps_acc.tile([128, 512], f32)
            den = ps_den.tile([1, 512], f32)

            def mm23_for(ptiles, ps2, g):
                for j in range(2):
                    lk = 2 * g + j
                    nc.tensor.matmul(
                        acc[:], v_sb[:, lk, :], ptiles[j][:]
                        if variant != "pefloor" else ptiles[j][:, 0, :],
                        start=(lk == 0), stop=(lk == NT - 1),
                        skip_group_check=True,
                    )
                if variant != "nomm3":
                    nc.tensor.matmul(
                        den[:], ones_r[:], ps2[:],
                        start=(g == 0), stop=(g == NG - 1),
                        skip_group_check=True,
                    )

            pipe = []
            for g in range(NG):
                ptiles = []
                for j in range(2):
                    lk = 2 * g + j
                    s1 = ps_s.tile([128, 512], f32, tag="s1", name="s1")
                    nc.tensor.matmul(
                        s1[:], kT_sb[:, lk * 128:(lk + 1) * 128],
                        qT_sb[:, cs], start=True, stop=True,
                    )
                    if variant == "pefloor":
                        ptiles.append(pconst)
                        snk = dsb.tile([128, 2], f32, tag="snk", name="snk")
                        nc.vector.tensor_copy(out=snk[:], in_=s1[:, 0:2])
                    else:
                        p1 = pexp.tile([128, 512], f32r, tag="p1", name="p1")
                        nc.scalar.activation(out=p1[:], in_=s1[:], func=AF.Exp)
                        ptiles.append(p1)
                # pre-sum the two 512-slices on DVE so the denominator
                # matmul streams half the columns
                ps2 = psum2p.tile([128, 512], f32r, tag="ps2", name="ps2")
                nc.vector.tensor_tensor(
                    out=ps2[:], in0=ptiles[0][:].bitcast(f32),
                    in1=ptiles[1][:].bitcast(f32), op=ALU.add,
                )
                pipe.append((ptiles, ps2, g))
                if len(pipe) > 2:
                    mm23_for(*pipe.pop(0))
            while pipe:
                mm23_for(*pipe.pop(0))

            # denominator row -> free dim of partition 0, then transpose to
            # per-partition scale columns
            tall = dsb.tile([1, 512], f32, tag="tall")
            nc.vector.tensor_copy(out=tall[:], in_=den[:])
            ps_sc = ps_sc_p.tile([128, 4], f32)
            for m in range(4):
                nc.tensor.transpose(
                    ps_sc[:, m:m + 1], tall[0:1, m * 128:(m + 1) * 128], id1[:]
                )
            sc_raw = dsb.tile([128, 4], f32, tag="scraw")
            nc.vector.tensor_copy(out=sc_raw[:], in_=ps_sc[:])
            sc = dsb.tile([128, 4], f32, tag="sc")
            nc.vector.reciprocal(out=sc[:], in_=sc_raw[:])
            nc.vector.tensor_scalar_mul(sc[:], sc[:], gamma_sb[:])

            # A~^T to SBUF (fp32r) for MM4
            a_sb = asb.tile([128, 512], f32r)
            nc.vector.tensor_copy(out=a_sb[:], in_=acc[:])

            o_sb = osb.tile([128, 4, C], f32)
            for m in range(4):
                t = 4 * c + m
                po = ps_out.tile([128, C], f32, tag="po")
                nc.tensor.matmul(
                    po[:], a_sb[:, m * 128:(m + 1) * 128], wl_r[:],
                    start=True, stop=True,
                )
                nc.vector.scalar_tensor_tensor(
                    out=o_sb[:, m, :], in0=po[:], scalar=sc[:, m:m + 1],
                    in1=x_sb[:, t, :], op0=ALU.mult, op1=ALU.add,
                )
            nc.sync.dma_start(
                out=out_tiled[:, 4 * c:4 * c + 4, :], in_=o_sb[:]
            )


_NC_CACHE = {}


def _build(nreps=1):
    """Build the Bass module; nreps>1 repeats the whole body (for marginal-
    time measurement in the dev harness — grading path uses nreps=1)."""
    if nreps not in _NC_CACHE:
        from contextlib import ExitStack

        nc = bacc.Bacc("TRN2", target_bir_lowering=False)
        with tile.TileContext(nc) as tc:
            with ExitStack() as ctx:
                _emit(nc, tc, ctx, nreps=nreps)
        nc.compile()
        _NC_CACHE[nreps] = nc
    return _NC_CACHE[nreps]


def kernel(x, Wq, Wk, Wv, Wlast, gamma):
    assert x.shape == (B, H, W, C), x.shape
    nc = _build()
    xf = np.ascontiguousarray(x, dtype=np.float32).reshape(B, L, C)
    in_maps = [
        {
            "x": xf[b],
            "Wq": np.ascontiguousarray(Wq, dtype=np.float32),
            "Wk": np.ascontiguousarray(Wk, dtype=np.float32),
            "Wv": np.ascontiguousarray(Wv, dtype=np.float32),
            "Wlast": np.ascontiguousarray(Wlast, dtype=np.float32),
            "gamma": np.ascontiguousarray(gamma, dtype=np.float32),
        }
        for b in range(B)
    ]
    res = run_bass_kernel_spmd(nc, in_maps, core_ids=list(range(B)))
    out = np.stack([res.results[b]["out"] for b in range(B)], axis=0)
    return out.reshape(B, H, W, C)
